# revision 12
# baseline (speedup 1.0000x reference)
"""Distributed GAT (2-layer, PyG GATConv semantics) as a Bass/Tile SPMD kernel
for 8 Trainium2 NeuronCores.

Sharding: nodes row-sharded across cores; edges sharded by dst. Per layer each
core computes table_local = x_local @ [W | W@As | W@Ad | 0pad] ([npc, 384]
bf16: h | alpha_src | alpha_dst | pad), AllGather -> full table [N, 384], with
adt := cols 256:384 of the local table (asrc|adst|pad). Edge phase per
SUPER-GROUP of SG=7 dst groups (edges dst-sorted, split into src<32768 (A) /
src>=32768 (B) sections for int16 dma_gather indices, each padded to a
core-uniform subtile count):
  dma_gather [h|asrc] rows by src (768B bf16 elems),
  dma_gather adt rows by local dst (256B bf16 elems),
  e = exp(leakyrelu(asrc+adst))  (softmax without max-subtraction: alphas are
      O(5) so exp is safe; mathematically identical),
  w = h * e (head-wise),      [one DVE instr per super-group per op]
  segment-sum via PE matmuls with on-device-built bf16 selector S
      (is_equal vs iota) into a PSUM group accumulator [128, 264],
  flush: x' = relu(sum_w / sum_e + bias) -> feeds next layer table (or head).
Final head: logits = x3 @ Wc + bc, log_softmax, row-sharded output.

All heavy dtypes are bf16 (platform cost is dominated by per-instruction
overhead + DMA bytes; bf16 halves gather/AllGather volume at equal
instruction cost). PSUM accumulation stays f32.
"""
import math
import numpy as np

import concourse.bass as bass
import concourse.bacc as bacc
import concourse.tile as tile
from concourse import mybir
from concourse.masks import make_identity

F32 = mybir.dt.float32
BF16 = mybir.dt.bfloat16
I16 = mybir.dt.int16

P = 128
SPLIT_AT = 32768          # int16 index limit
SG_HOST = 7               # groups per super-group (must match build sg=)          # int16 index limit for dma_gather


class Cfg:
    def __init__(self, N, DIN, H, C, OUT, n_cores):
        self.N, self.DIN, self.H, self.C, self.OUT = N, DIN, H, C, OUT
        self.HID = H * C
        self.GC = self.HID + H              # useful gathered cols: h | asrc
        self.RT = 384                       # bf16 table row: 768B % 256 == 0
        self.AC = 128                       # adt row (256B bf16)
        assert self.HID + 2 * H <= self.RT
        self.n_cores = n_cores
        assert N % n_cores == 0
        self.npc = N // n_cores
        assert self.npc <= SPLIT_AT, "local dst must fit int16"
        self.NB = math.ceil(self.npc / P)
        self.npc_pad = self.NB * P
        self.split = N > SPLIT_AT
        # filled by preprocess:
        self.UA = None
        self.UB = None

    @property
    def UT(self):
        return self.UA + self.UB


def _wrap_idx(vals):
    """int16 index list (len % 16 == 0) -> dma_gather wrapped layout
    [128, len/16]: index j at partition j%16 col j//16, replicated x8."""
    n = len(vals)
    w = vals.reshape(n // 16, 16).T.astype(np.int16)   # [16, n/16]
    return np.tile(w, (8, 1))                          # [128, n/16]


def preprocess(cfg: Cfg, edge_index: np.ndarray):
    """Per-core edge-stream arrays for the group-wise dma_gathers.

    Returns list per core of dict:
      idxA [NB, 128, UA*8] i16   (src < SPLIT_AT)
      idxB [NB, 128, UB*8] i16   (src - SPLIT_AT)   (only if cfg.split)
      idxD [NB, 128, UT*8] i16   (local dst, for adt)
      dstc [NB, 128, UT]   f32   (dst rel to group, -1 for padding)
    """
    N, n_cores, npc = cfg.N, cfg.n_cores, cfg.npc
    NB, npc_pad = cfg.NB, cfg.npc_pad

    src = np.concatenate([edge_index[0], np.arange(N, dtype=edge_index.dtype)])
    dst = np.concatenate([edge_index[1], np.arange(N, dtype=edge_index.dtype)])
    order = np.argsort(dst, kind="stable")
    src_s = np.asarray(src[order], dtype=np.int64)
    dst_s = np.asarray(dst[order], dtype=np.int64)
    bounds = np.searchsorted(dst_s, np.arange(n_cores + 1) * npc)

    cores = []
    UA = UB = 0
    for c in range(n_cores):
        lo, hi = bounds[c], bounds[c + 1]
        s_c = src_s[lo:hi]
        d_c = dst_s[lo:hi] - c * npc
        if npc_pad > npc:  # fake dst rows so every psum row has a real denom
            fake = np.arange(npc, npc_pad, dtype=np.int64)
            s_c = np.concatenate([s_c, np.zeros(len(fake), np.int64)])
            d_c = np.concatenate([d_c, fake])
        isB = (s_c >= SPLIT_AT) if cfg.split else np.zeros(len(s_c), bool)
        g_c = d_c // P
        # sort by (group, section, dst)
        key = (g_c * 2 + isB) * npc_pad + d_c
        o = np.argsort(key, kind="stable")
        s_c, d_c, g_c, isB = s_c[o], d_c[o], g_c[o], isB[o]
        cntA = np.bincount(g_c[~isB], minlength=NB)
        cntB = np.bincount(g_c[isB], minlength=NB)
        UA = max(UA, int(math.ceil(cntA.max() / P)))
        if cfg.split:
            UB = max(UB, int(math.ceil(cntB.max() / P)))
        cores.append((s_c, d_c, g_c, isB, cntA, cntB))
    cfg.UA, cfg.UB = UA, UB
    UT = UA + UB

    out = []
    for (s_c, d_c, g_c, isB, cntA, cntB) in cores:
        # target slot within the group stream [A pad to UA*128 | B pad to UB*128]
        startA = np.zeros(NB + 1, np.int64)
        np.cumsum(cntA, out=startA[1:])
        startB = np.zeros(NB + 1, np.int64)
        np.cumsum(cntB, out=startB[1:])
        rank = np.empty(len(d_c), np.int64)
        idxall = np.arange(len(d_c), dtype=np.int64)
        # positions: edges are sorted (group, section); rank within section:
        secA = ~isB
        rank[secA] = idxall[secA] - (startA[g_c[secA]] + startB[g_c[secA]])
        rank[isB] = idxall[isB] - (startA[g_c[isB] + 1] + startB[g_c[isB]])
        tgt = g_c * (UT * P) + np.where(isB, UA * P + rank, rank)

        srcv = np.zeros(NB * UT * P, np.int64)          # pad -> row 0
        dstl = np.zeros(NB * UT * P, np.int64)          # pad -> row 0
        dstcv = np.full(NB * UT * P, -1.0, np.float32)  # pad -> no dst
        srcv[tgt] = np.where(isB, s_c - SPLIT_AT, s_c)
        dstl[tgt] = np.where(d_c < npc, d_c, 0)         # fake dsts gather row 0
        dstcv[tgt] = (d_c % P).astype(np.float32)

        srcv = srcv.reshape(NB, UT * P)
        dstl = dstl.reshape(NB, UT * P)
        dstcv = dstcv.reshape(NB, UT, P)

        idxA = np.stack([_wrap_idx(srcv[g, :UA * P]) for g in range(NB)])
        idxD = np.stack([_wrap_idx(dstl[g]) for g in range(NB)])
        d = {
            "idxA": idxA.astype(np.int16),
            "idxD": idxD.astype(np.int16),
            # dstc[g, p, k] = value at stream pos k*128+p
            "dstc": np.ascontiguousarray(dstcv.transpose(0, 2, 1)),
        }
        if cfg.split:
            idxB = np.stack([_wrap_idx(srcv[g, UA * P:]) for g in range(NB)])
            d["idxB"] = idxB.astype(np.int16)
        out.append(d)
    return out


def expand_att(a, HID, H, C):
    A = np.zeros((HID, H), np.float32)
    for h in range(H):
        A[h * C:(h + 1) * C, h] = a[h]
    return A


def build_program(cfg: Cfg, edge_reps=1, no_collectives=False, pert=frozenset(),
                  gw_bufs=1, idx_bufs=1, sq=4, sg=7, ch=8):
    """Emit the (core-uniform) SPMD program. Returns nc."""
    NB, UA, UB, UT = cfg.NB, cfg.UA, cfg.UB, cfg.UT
    RT, GC, AC = cfg.RT, cfg.GC, cfg.AC
    HID, OUT, DIN, H = cfg.HID, cfg.OUT, cfg.DIN, cfg.H
    npc, N = cfg.npc, cfg.N
    DC = DIN // P
    HC = HID // P
    NA_ROWS = min(N, SPLIT_AT)
    SG = sg
    assert NB % SG == 0, (NB, SG)

    nc = bacc.Bacc("TRN2", target_bir_lowering=False, debug=False,
                   num_devices=cfg.n_cores, num_swdge_queues=sq)

    t_xT = nc.dram_tensor("xT", [DIN, npc], BF16, kind="ExternalInput")
    t_M1 = nc.dram_tensor("M1", [DIN, RT], BF16, kind="ExternalInput")
    t_M2 = nc.dram_tensor("M2", [HID, RT], BF16, kind="ExternalInput")
    t_Wc = nc.dram_tensor("Wc", [HID, OUT], BF16, kind="ExternalInput")
    t_b1 = nc.dram_tensor("b1", [P, HID], BF16, kind="ExternalInput")
    t_b2 = nc.dram_tensor("b2", [P, HID], BF16, kind="ExternalInput")
    t_bc = nc.dram_tensor("bc", [P, OUT], F32, kind="ExternalInput")
    t_iota = nc.dram_tensor("iota", [P, P], F32, kind="ExternalInput")
    NS = NB // SG
    t_idxA = nc.dram_tensor("idxA", [NS, P, SG * UA * 8], I16,
                            kind="ExternalInput")
    if cfg.split:
        t_idxB = nc.dram_tensor("idxB", [NS, P, SG * UB * 8], I16,
                                kind="ExternalInput")
    t_idxD = nc.dram_tensor("idxD", [NS, P, SG * UT * 8], I16,
                            kind="ExternalInput")
    t_dstc = nc.dram_tensor("dstc", [NS, P, SG * UT], F32,
                            kind="ExternalInput")
    t_out = nc.dram_tensor("out", [cfg.npc_pad, OUT], F32,
                           kind="ExternalOutput")

    rgroups = [list(range(cfg.n_cores))]

    with tile.TileContext(nc) as tc:
        with (
            tc.tile_pool(name="const", bufs=1) as cp,
            tc.tile_pool(name="xt", bufs=1) as xtp,
            tc.tile_pool(name="tbl", bufs=1) as tblp,
            tc.tile_pool(name="gw", bufs=gw_bufs) as gwp,
            tc.tile_pool(name="idx", bufs=idx_bufs) as idxp,
            tc.tile_pool(name="ad", bufs=gw_bufs) as adp,
            tc.tile_pool(name="zz", bufs=1) as zzp,
            tc.tile_pool(name="smat", bufs=gw_bufs) as sp,
            tc.tile_pool(name="xb", bufs=1) as xbp,
            tc.tile_pool(name="xtb", bufs=1) as xtbp,
            tc.tile_pool(name="hd", bufs=1) as hdp,
            tc.tile_pool(name="ps_acc", bufs=2, space="PSUM") as ps_acc,
            tc.tile_pool(name="ps_tp", bufs=2, space="PSUM") as ps_tp,
            tc.tile_pool(name="ps_tb", bufs=2, space="PSUM") as ps_tb,
            tc.tile_pool(name="ps_lg", bufs=2, space="PSUM") as ps_lg,
            tc.tile_pool(name="dram", bufs=1, space="DRAM") as dp,
        ):
            # ---- constants ----
            M1sb = cp.tile([P, DC, RT], BF16)
            M2sb = cp.tile([P, HC, RT], BF16)
            WcSb = cp.tile([P, HC, OUT], BF16)
            b1sb = cp.tile([P, HID], BF16)
            b2sb = cp.tile([P, HID], BF16)
            bcsb = cp.tile([P, OUT], F32)
            iota = cp.tile([P, P], F32)
            identb = cp.tile([P, P], BF16)
            nc.sync.dma_start(out=M1sb[:], in_=t_M1[:, :].rearrange(
                "(a c) r -> c a r", c=P))
            nc.sync.dma_start(out=M2sb[:], in_=t_M2[:, :].rearrange(
                "(a c) r -> c a r", c=P))
            nc.sync.dma_start(out=WcSb[:], in_=t_Wc[:, :].rearrange(
                "(a c) r -> c a r", c=P))
            nc.sync.dma_start(out=b1sb[:], in_=t_b1[:, :])
            nc.sync.dma_start(out=b2sb[:], in_=t_b2[:, :])
            nc.sync.dma_start(out=bcsb[:], in_=t_bc[:, :])
            nc.sync.dma_start(out=iota[:], in_=t_iota[:, :])
            make_identity(nc, identb[:])

            # ---- internal DRAM ----
            ag1_in = dp.tile([cfg.npc_pad, RT], BF16)
            ag2_in = dp.tile([cfg.npc_pad, RT], BF16)
            table1 = dp.tile([N, RT], BF16)
            table2 = dp.tile([N, RT], BF16)

            # ---- phase B: layer-1 table (writes batched per SG blocks) ----
            for b in range(NB):
                ncols = min(P, npc - b * P)
                xt = xtp.tile([P, DC, P], BF16, name="xt")
                nc.sync.dma_start(
                    out=xt[:, :, 0:ncols],
                    in_=t_xT[:, b * P:b * P + ncols].rearrange(
                        "(a c) n -> c a n", c=P))
                pstb = ps_tb.tile([P, RT], F32, name="pstb", tag="pstb")
                for a in range(DC):
                    nc.tensor.matmul(pstb[0:ncols, :], xt[:, a, 0:ncols],
                                     M1sb[:, a, :], start=(a == 0),
                                     stop=(a == DC - 1))
                if b % SG == 0:
                    tbsS1 = tblp.tile([P, SG, RT], BF16, name="tbsS")
                nc.vector.tensor_copy(out=tbsS1[:, b % SG, :], in_=pstb[:])
                if b % SG == SG - 1:
                    b0 = b - (SG - 1)
                    nc.sync.dma_start(
                        out=ag1_in[b0 * P:(b + 1) * P, :].rearrange(
                            "(s p) r -> p s r", p=P),
                        in_=tbsS1[:])

            if no_collectives:
                nc.sync.dma_start(out=table1[0:npc, :], in_=ag1_in[0:npc, :])
            else:
                nc.gpsimd.collective_compute(
                    "AllGather", mybir.AluOpType.bypass, replica_groups=rgroups,
                    ins=[ag1_in[0:npc, :].opt()], outs=[table1[:].opt()])

            # ---- edge phase (shared by both layers) ----
            def edge_phase(table_full, ag_local, t_idxB, flush_fn):
                qn = [0]
                CH = ch  # subtiles per dma_gather (ucode ring: <=1024 idxs)

                def chunked_gather(dst_view, src_ap, idx_view, u, elem,
                                   estep=None):
                    for c0 in range(0, u, CH):
                        c1 = min(c0 + CH, u)
                        n = (c1 - c0) * P
                        qn[0] = (qn[0] + 1) % sq
                        if "densegather" in pert:
                            nc.sync.dma_start(
                                out=dst_view[:, c0:c1, :],
                                in_=src_ap.tensor.ap()[0:n, 0:elem].rearrange(
                                    "(k p) r -> p k r", p=P))
                        else:
                            nc.gpsimd.dma_gather(
                                dst_view[:, c0:c1, :], src_ap,
                                idx_view[:, c0 * 8:c1 * 8], n, n, elem,
                                elem_step=estep, queue_num=qn[0])

                for g0 in range(0, NB, SG):
                    sb = g0 // SG
                    # ---- batched idx loads (host pre-batched layouts) ----
                    ia = idxp.tile([P, SG, UA * 8], I16, name="ia")
                    nc.sync.dma_start(
                        out=ia[:].rearrange("p s w -> p (s w)"),
                        in_=t_idxA[sb])
                    idt = idxp.tile([P, SG, UT * 8], I16, name="idt")
                    nc.sync.dma_start(
                        out=idt[:].rearrange("p s w -> p (s w)"),
                        in_=t_idxD[sb])
                    dc = idxp.tile([P, SG, UT], F32, name="dc")
                    nc.sync.dma_start(
                        out=dc[:].rearrange("p s w -> p (s w)"),
                        in_=t_dstc[sb])
                    if cfg.split:
                        ib = idxp.tile([P, SG, UB * 8], I16, name="ib")
                        nc.sync.dma_start(
                            out=ib[:].rearrange("p s w -> p (s w)"),
                            in_=t_idxB[sb])

                    gAs = gwp.tile([P, SG, UA, RT], BF16, name="gAs")
                    gBs = (gwp.tile([P, SG, UB, RT], BF16, name="gBs")
                           if cfg.split else None)
                    ads = adp.tile([P, SG, UT, AC], BF16, name="ads")
                    if "nomain" not in pert:
                        chunked_gather(
                            gAs[:].rearrange("p s k r -> p (s k) r"),
                            table_full[0:NA_ROWS, :],
                            ia[:].rearrange("p s w -> p (s w)"), SG * UA, RT)
                        if cfg.split:
                            chunked_gather(
                                gBs[:].rearrange("p s k r -> p (s k) r"),
                                table_full[SPLIT_AT:N, :],
                                ib[:].rearrange("p s w -> p (s w)"),
                                SG * UB, RT)
                    if "noadt" not in pert:
                        # adst straight from the local table rows (cols
                        # HID:HID+AC), elem_step = full row stride
                        chunked_gather(
                            ads[:].rearrange("p s k r -> p (s k) r"),
                            ag_local[:, HID:HID + AC],
                            idt[:].rearrange("p s w -> p (s w)"),
                            SG * UT, AC, estep=RT)

                    # e = exp(lrelu(asrc + adst)); w = h * e   [super-group ops]
                    zts = zzp.tile([P, SG, UT, H], F32, name="zts")
                    nc.vector.tensor_add(
                        out=zts[:, :, 0:UA, :],
                        in0=gAs[:, :, :, HID:GC],
                        in1=ads[:, :, 0:UA, H:2 * H])
                    if cfg.split:
                        nc.vector.tensor_add(
                            out=zts[:, :, UA:UT, :],
                            in0=gBs[:, :, :, HID:GC],
                            in1=ads[:, :, UA:UT, H:2 * H])
                    zf = zts[:].rearrange("p s k h -> p (s k) h")
                    nc.vector.scalar_tensor_tensor(
                        out=zf, in0=zf, scalar=0.2, in1=zf,
                        op0=mybir.AluOpType.mult, op1=mybir.AluOpType.max)
                    nc.scalar.activation(gAs[:, :, :, HID:GC],
                                         zts[:, :, 0:UA, :],
                                         mybir.ActivationFunctionType.Exp)
                    if cfg.split:
                        nc.scalar.activation(gBs[:, :, :, HID:GC],
                                             zts[:, :, UA:UT, :],
                                             mybir.ActivationFunctionType.Exp)
                    if "nowmul" not in pert:
                        for gt, u in (((gAs, UA), (gBs, UB)) if cfg.split
                                      else ((gAs, UA),)):
                            flat = gt[:].rearrange("p s k r -> p (s k) r")
                            e_b = flat[:, :, HID:GC].to_broadcast(
                                [P, SG * u, H, cfg.C])
                            hv = flat[:, :, 0:HID].rearrange(
                                "p k (h c) -> p k h c", c=cfg.C)
                            nc.vector.tensor_mul(out=hv, in0=hv, in1=e_b)
                    Sts = sp.tile([P, SG, UT, P], BF16, name="Sts")
                    if "nosbuild" not in pert:
                        nc.vector.tensor_tensor(
                            out=Sts[:],
                            in0=dc[:].to_broadcast([P, SG, UT, P]),
                            in1=iota[:].rearrange("p (a o i) -> p a o i",
                                                  a=1, o=1)
                            .to_broadcast([P, SG, UT, P]),
                            op=mybir.AluOpType.is_equal)
                    accS = xbp.tile([P, SG, GC], F32, name="accS")
                    for s in range(SG):
                        acc = ps_acc.tile([P, GC], F32, name="acc")
                        for j in range(UT):
                            gt, k = (gAs, j) if j < UA else (gBs, j - UA)
                            nc.tensor.matmul(acc[:], Sts[:, s, j, :],
                                             gt[:, s, k, 0:GC],
                                             start=(j == 0),
                                             stop=(j == UT - 1))
                        nc.vector.tensor_copy(out=accS[:, s, :], in_=acc[:])
                    if "noflush" not in pert:
                        flush_fn(g0, accS)

            # ---- flush helpers (batched over SG groups) ----
            def normalizeS(accS, bias_sb):
                recS = zzp.tile([P, SG, H], F32, name="recS")
                nc.vector.reciprocal(recS[:], accS[:, :, HID:GC])
                xbS = xbp.tile([P, SG, HID], BF16, name="xbS")
                nc.vector.tensor_mul(
                    out=xbS[:].rearrange("p s (h c) -> p s h c", c=cfg.C),
                    in0=accS[:, :, 0:HID].rearrange("p s (h c) -> p s h c",
                                                    c=cfg.C),
                    in1=recS[:].to_broadcast([P, SG, H, cfg.C]))
                nc.vector.tensor_add(
                    out=xbS[:], in0=xbS[:],
                    in1=bias_sb[:].rearrange("p (o w) -> p o w", o=1)
                    .to_broadcast([P, SG, HID]))
                nc.vector.tensor_scalar_max(xbS[:], xbS[:], 0.0)
                return xbS

            def transpose2(xb):
                xts2 = xtbp.tile([P, HC, P], BF16, name="xts2")
                for a in range(HC):
                    pst = ps_tp.tile([P, P], BF16, name="pst")
                    nc.tensor.transpose(pst[:], xb[:, a * P:(a + 1) * P],
                                        identb[:])
                    nc.vector.tensor_copy(out=xts2[:, a, :], in_=pst[:])
                return xts2

            def flush_layer1(g0, accS):
                xbS = normalizeS(accS, b1sb)
                tbsS = tblp.tile([P, SG, RT], BF16, name="tbsS")
                for s in range(SG):
                    xts2 = transpose2(xbS[:, s, :])
                    pstb = ps_tb.tile([P, RT], F32, name="pstb2", tag="pstb")
                    for a in range(HC):
                        nc.tensor.matmul(pstb[:], xts2[:, a, :],
                                         M2sb[:, a, :], start=(a == 0),
                                         stop=(a == HC - 1))
                    nc.vector.tensor_copy(out=tbsS[:, s, :], in_=pstb[:])
                nc.sync.dma_start(
                    out=ag2_in[g0 * P:(g0 + SG) * P, :].rearrange(
                        "(s p) r -> p s r", p=P),
                    in_=tbsS[:])

            def flush_layer2(g0, accS):
                xbS = normalizeS(accS, b2sb)
                lgS = hdp.tile([P, SG, OUT], F32, name="lgS")
                for s in range(SG):
                    xts2 = transpose2(xbS[:, s, :])
                    pslg = ps_lg.tile([P, OUT], F32, name="pslg")
                    for a in range(HC):
                        nc.tensor.matmul(pslg[:], xts2[:, a, :],
                                         WcSb[:, a, :], start=(a == 0),
                                         stop=(a == HC - 1))
                    nc.vector.tensor_copy(out=lgS[:, s, :], in_=pslg[:])
                nc.vector.tensor_add(
                    out=lgS[:], in0=lgS[:],
                    in1=bcsb[:, 0:OUT].rearrange("p (o w) -> p o w", o=1)
                    .to_broadcast([P, SG, OUT]))
                mxS = hdp.tile([P, SG, 1], F32, name="mxS")
                nc.vector.tensor_reduce(out=mxS[:], in_=lgS[:],
                                        axis=mybir.AxisListType.X,
                                        op=mybir.AluOpType.max)
                nc.vector.tensor_sub(out=lgS[:], in0=lgS[:],
                                     in1=mxS[:].to_broadcast([P, SG, OUT]))
                exS = accS[:, :, 0:OUT]
                nc.scalar.activation(exS, lgS[:],
                                     mybir.ActivationFunctionType.Exp)
                dnS = hdp.tile([P, SG, 1], F32, name="dnS")
                nc.vector.tensor_reduce(out=dnS[:], in_=exS,
                                        axis=mybir.AxisListType.X,
                                        op=mybir.AluOpType.add)
                lndS = hdp.tile([P, SG, 1], F32, name="lndS")
                nc.scalar.activation(lndS[:], dnS[:],
                                     mybir.ActivationFunctionType.Ln)
                nc.vector.tensor_sub(out=lgS[:], in0=lgS[:],
                                     in1=lndS[:].to_broadcast([P, SG, OUT]))
                nc.sync.dma_start(
                    out=t_out[g0 * P:(g0 + SG) * P, :].rearrange(
                        "(s p) o -> p s o", p=P),
                    in_=lgS[:])

            for _ in range(edge_reps):
                edge_phase(table1, ag1_in, t_idxB if cfg.split else None,
                           flush_layer1)
            if no_collectives:
                nc.sync.dma_start(out=table2[0:npc, :], in_=ag2_in[0:npc, :])
            else:
                nc.gpsimd.collective_compute(
                    "AllGather", mybir.AluOpType.bypass, replica_groups=rgroups,
                    ins=[ag2_in[0:npc, :].opt()], outs=[table2[:].opt()])
            for _ in range(edge_reps):
                edge_phase(table2, ag2_in, t_idxB if cfg.split else None,
                           flush_layer2)

    nc.compile()
    return nc


def make_in_maps(cfg: Cfg, pre, x, W1, as1, ad1, b1, W2, as2, ad2, b2, Wc, bc):
    import ml_dtypes
    bf16 = ml_dtypes.bfloat16
    H, C, HID, npc, RT = cfg.H, cfg.C, cfg.HID, cfg.npc, cfg.RT

    def mk_m(W, a_s, a_d):
        M = np.zeros((W.shape[0], RT), np.float32)
        M[:, 0:HID] = W
        M[:, HID:HID + H] = W @ expand_att(a_s, HID, H, C)
        M[:, HID + H:HID + 2 * H] = W @ expand_att(a_d, HID, H, C)
        return M.astype(bf16)

    M1 = mk_m(W1, as1, ad1)
    M2 = mk_m(W2, as2, ad2)
    iota = np.tile(np.arange(P, dtype=np.float32)[None, :], (P, 1))
    maps = []
    for c in range(cfg.n_cores):
        def bat(a):
            # [NB, P, W] -> [NB//SG, P, SG*W] (contiguous super-group loads)
            NBl, Pl, W = a.shape
            return np.ascontiguousarray(
                a.reshape(NBl // SG_HOST, SG_HOST, Pl, W)
                .transpose(0, 2, 1, 3).reshape(NBl // SG_HOST, Pl,
                                               SG_HOST * W))
        m = {
            "xT": np.ascontiguousarray(x[c * npc:(c + 1) * npc].T).astype(bf16),
            "M1": M1, "M2": M2, "Wc": Wc.astype(bf16),
            "b1": np.tile(b1[None, :], (P, 1)).astype(bf16),
            "b2": np.tile(b2[None, :], (P, 1)).astype(bf16),
            "bc": np.tile(bc[None, :], (P, 1)).astype(np.float32),
            "iota": iota,
            "idxA": bat(pre[c]["idxA"]), "idxD": bat(pre[c]["idxD"]),
            "dstc": bat(pre[c]["dstc"].reshape(cfg.NB, P, -1)),
        }
        if cfg.split:
            m["idxB"] = bat(pre[c]["idxB"])
        maps.append(m)
    return maps


# ---------------------------------------------------------------------------
# Harness entry point: full inputs in, full output out.
# ---------------------------------------------------------------------------

def kernel(x, edge_index, W1, as1, ad1, b1, W2, as2, ad2, b2, Wc, bc):
    x = np.asarray(x, dtype=np.float32)
    edge_index = np.asarray(edge_index)
    N, DIN = x.shape
    H, C = np.asarray(as1).shape
    OUT = np.asarray(Wc).shape[1]
    n_cores = 8

    cfg = Cfg(N, DIN, H, C, OUT, n_cores)
    pre = preprocess(cfg, edge_index)
    nc = build_program(cfg)
    in_maps = make_in_maps(cfg, pre, x,
                           np.asarray(W1, np.float32), np.asarray(as1, np.float32),
                           np.asarray(ad1, np.float32), np.asarray(b1, np.float32),
                           np.asarray(W2, np.float32), np.asarray(as2, np.float32),
                           np.asarray(ad2, np.float32), np.asarray(b2, np.float32),
                           np.asarray(Wc, np.float32), np.asarray(bc, np.float32))

    from concourse import bass_utils
    last_err = None
    for _attempt in range(3):   # a wedged device from a prior crash can fail once
        try:
            res = bass_utils.run_bass_kernel_spmd(nc, in_maps,
                                                  core_ids=list(range(n_cores)))
            break
        except Exception as e:                      # noqa: BLE001
            last_err = e
    else:
        raise last_err
    return np.concatenate([res.results[c]["out"][:cfg.npc]
                           for c in range(n_cores)], axis=0)
